# revision 20
# baseline (speedup 1.0000x reference)
"""AGCRN cell Bass kernel for 8 TRN2 NeuronCores.

Sharding: data-parallel over batch (B=8 -> 1 batch per core). Zero collectives.

Math (per core, batch b):
  S_un = exp(relu(E @ E.T))           # symmetric! (elementwise funcs of symmetric EE^T)
  r    = 1 / rowsum(S_un)             # softmax normalization deferred to row scales
  S1 @ v = diag(r) @ (S_un @ v)       # S_un used as matmul weights directly (no transpose
                                      #   needed thanks to symmetry)
  Chebyshev by recurrence on the activation side:
    G1 = S1 @ X;  G2 = 2*S1@G1 - X    (never form S2 = 2 S1 S1 - I)
  Weight application without materializing per-node weights:
    y[n,(d,o)] = x_g[n,(k,i)] @ W2[(k,i),(d,o)],  W2[(k,i),(d,o)] = pool[d,k,i,o]
    out[n,o]   = sum_d E[n,d] * y[n,(d,o)] (+bias via augmented ones-row in x_g/W2)
  E-contraction on DVE via scalar_tensor_tensor with per-partition scalars E[n,d].
"""

import numpy as np
import ml_dtypes

B, N, D, CIN, H = 8, 2048, 16, 2, 64
C = CIN + H          # 66
KI = 3 * C + 1       # 199 = (k,i) rows + 1 bias ones-row
DOG = D * 2 * H      # 2048 gate (d,o) cols
DOU = D * H          # 1024 update
P = 128
NT = N // P          # 16

_CACHE = {}
_EYE = np.eye(P, dtype=ml_dtypes.bfloat16)


def _build():
    import concourse.bass as bass
    import concourse.tile as tile
    import concourse.mybir as mybir
    from concourse import bacc

    f32 = mybir.dt.float32
    bf16 = mybir.dt.bfloat16
    AF = mybir.ActivationFunctionType
    OP = mybir.AluOpType

    nc = bacc.Bacc(None, target_bir_lowering=False)

    x_d = nc.declare_dram_parameter("x", [N, CIN], f32, isOutput=False)
    st_d = nc.declare_dram_parameter("state", [N, H], f32, isOutput=False)
    e_d = nc.declare_dram_parameter("emb", [N, D], f32, isOutput=False)
    id_d = nc.declare_dram_parameter("idn", [P, P], bf16, isOutput=False)
    wg_d = nc.declare_dram_parameter("w2g", [2, P, DOG], bf16, isOutput=False)
    wu_d = nc.declare_dram_parameter("w2u", [2, P, DOU], bf16, isOutput=False)
    out_d = nc.declare_dram_parameter("out", [N, H], f32, isOutput=True)

    with tile.TileContext(nc) as tc:
        import contextlib
        ctx = contextlib.ExitStack()
        with ctx:
            cpool = ctx.enter_context(tc.tile_pool(name="const", bufs=1))
            wpool = ctx.enter_context(tc.tile_pool(name="work", bufs=4))
            pp = ctx.enter_context(tc.tile_pool(name="psA", bufs=2, space="PSUM"))
            ppy = ctx.enter_context(tc.tile_pool(name="psY", bufs=1, space="PSUM"))
            papp = ctx.enter_context(tc.tile_pool(name="psB", bufs=2, space="PSUM"))

            # ---- persistent SBUF ----
            S = cpool.tile([P, NT, N], bf16)          # S_un, row-tile t on axis1
            Et = cpool.tile([D, N], bf16)             # E^T (d on partitions)
            En = cpool.tile([P, NT, D], f32)          # E natural (per-partition scalars)
            rinv = cpool.tile([P, NT], f32)
            rinv2 = cpool.tile([P, NT], f32)
            rsum = cpool.tile([P, NT], f32)
            xg = cpool.tile([P, NT, 256], bf16)       # x_g node-major cols 0:199 (+pad)
            xgT0 = cpool.tile([P, NT, P], bf16)       # (x_g)^T rows 0:128
            xgT1 = cpool.tile([P, NT, P], bf16)       # (x_g)^T rows 128:199 (71 used)
            w2g = cpool.tile([P, 2, DOG], bf16)
            w2u = cpool.tile([P, 2, DOU], bf16)
            st_f = cpool.tile([P, NT, H], f32)
            x_f = cpool.tile([P, NT, CIN], f32)
            z_f = cpool.tile([P, NT, H], f32)
            idn = cpool.tile([P, P], bf16)
            idn_st = cpool.tile([P, P], bf16)
            enb = cpool.tile([P, NT, D], bf16)
            wg_st = cpool.tile([P, 2, DOG], bf16)
            wu_st = cpool.tile([P, 2, DOU], bf16)

            # ---- DMA inputs ----
            nc.sync.dma_start(x_f[:], x_d.ap().rearrange("(t p) c -> p t c", p=P))
            nc.sync.dma_start(st_f[:], st_d.ap().rearrange("(t p) h -> p t h", p=P))
            nc.sync.dma_start(En[:], e_d.ap().rearrange("(t p) d -> p t d", p=P))
            nc.sync.dma_start(wg_st[:], wg_d.ap().rearrange("c p f -> p c f"))
            nc.sync.dma_start(wu_st[:], wu_d.ap().rearrange("c p f -> p c f"))
            nc.sync.dma_start(idn_st[:], id_d.ap())
            # funnel all PE-visible tensors through DVE (matmul 1-wait limit)
            nc.vector.tensor_copy(w2g[:], wg_st[:])
            nc.vector.tensor_copy(w2u[:], wu_st[:])
            nc.vector.tensor_copy(idn[:], idn_st[:])
            nc.vector.tensor_copy(enb[:], En[:])

            # x_g cols 0:2 = x, ones col at 198
            nc.vector.tensor_copy(xg[:, :, 0:CIN], x_f[:])
            nc.vector.tensor_copy(xg[:, :, CIN:C], st_f[:])
            nc.vector.memset(xg[:, :, 198:199], 1.0)

            # ---- E^T via TensorE transpose (bf16) ----
            for t in range(NT):
                pt = pp.tile([D, P], bf16, tag="tp")
                nc.tensor.transpose(pt[:], enb[:, t, :], idn[:, :])
                nc.vector.tensor_copy(Et[:, t * P:(t + 1) * P], pt[:])

            # ---- S_un build: EE^T (bf16 matmul) -> exp (ACT) -> max(1,.)+rowsum (DVE) ----
            Etb = Et  # already bf16
            rsum2 = cpool.tile([P, NT, 2], f32)
            for t in range(NT):
                for h in range(2):
                    ee = ppy.tile([P, DOG // 2], f32, tag="y")
                    for j in range(2):
                        nc.tensor.matmul(
                            ee[:, j * 512:(j + 1) * 512],
                            Etb[:, t * P:(t + 1) * P],
                            Etb[:, (2 * h + j) * 512:(2 * h + j + 1) * 512],
                            start=True, stop=True,
                        )
                    etmp = wpool.tile([P, N // 2], bf16, tag="etmp")
                    nc.scalar.activation(etmp[:], ee[:], AF.Exp)
                    nc.vector.tensor_scalar(
                        S[:, t, h * (N // 2):(h + 1) * (N // 2)], etmp[:],
                        1.0, 0.0, OP.max, OP.add,
                        accum_out=rsum2[:, t, h:h + 1],
                    )
            nc.vector.tensor_tensor(rsum[:], rsum2[:, :, 0], rsum2[:, :, 1],
                                    OP.add)
            nc.vector.reciprocal(rinv[:], rsum[:])
            nc.vector.tensor_scalar_mul(rinv2[:], rinv[:], 2.0)

            # ---- two phases: gate (sigmoid -> z, r) and update (tanh -> hc -> h) ----
            for phase in range(2):
                dox = DOG if phase == 0 else DOU
                w2 = w2g if phase == 0 else w2u

                # Setup B apps: X stationary, S streaming (few big MMs).
                # (S_un @ V)^T = V^T S_un by symmetry -> psum (C, N-half),
                # then per-tile PE transpose back to node-major xg cols.
                for app in range(2):
                    src_lo = 0 if app == 0 else C
                    gt = wpool.tile([C, N], bf16, tag="gt")
                    for h in range(2):
                        ap = papp.tile([C, N // 2], f32, tag="app")
                        for mc in range(NT):
                            for j in range(2):
                                nc.tensor.matmul(
                                    ap[:, j * 512:(j + 1) * 512],
                                    xg[:, mc, src_lo:src_lo + C],
                                    S[:, mc, h * 1024 + j * 512:
                                      h * 1024 + (j + 1) * 512],
                                    start=(mc == 0), stop=(mc == NT - 1),
                                )
                        nc.scalar.copy(gt[:, h * 1024:(h + 1) * 1024], ap[:])
                    for t in range(NT):
                        pt = pp.tile([P, C], bf16, tag="tp")
                        nc.tensor.transpose(
                            pt[:], gt[:, t * P:(t + 1) * P], idn[:66, :66])
                        if app == 0:
                            nc.vector.tensor_scalar(
                                xg[:, t, C:2 * C], pt[:], rinv[:, t:t + 1],
                                None, OP.mult,
                            )
                        else:
                            nc.vector.scalar_tensor_tensor(
                                xg[:, t, 2 * C:3 * C], pt[:], rinv2[:, t:t + 1],
                                xg[:, t, 0:C], OP.mult, OP.subtract,
                            )

                for t in range(NT):
                    # transpose x_g tile -> (ki, n) chunks
                    pt0 = pp.tile([P, P], bf16, tag="tp")
                    nc.tensor.transpose(pt0[:], xg[:, t, 0:P], idn[:, :])
                    nc.vector.tensor_copy(xgT0[:, t, :], pt0[:])
                    pt1 = pp.tile([71, P], bf16, tag="tp")
                    nc.tensor.transpose(pt1[:], xg[:, t, P:KI], idn[:, :])
                    nc.vector.tensor_copy(xgT1[:71, t, :], pt1[:])

                    # weight matmul in halves: ACT copy of half h overlaps
                    # PE matmul of half h+1 (separate psum tiles, bufs=2)
                    oo = dox // D  # 128 gate, 64 update
                    ysb = wpool.tile([P, oo, D], bf16, tag="ysb")
                    hw_ = dox // 2
                    for h in range(2):
                        y = ppy.tile([P, hw_], f32, tag="y")
                        for j in range(hw_ // 512):
                            jj = h * (hw_ // 512) + j
                            nc.tensor.matmul(
                                y[:, j * 512:(j + 1) * 512], xgT0[:, t, :],
                                w2[:, 0, jj * 512:(jj + 1) * 512],
                                start=True, stop=False,
                            )
                            nc.tensor.matmul(
                                y[:, j * 512:(j + 1) * 512], xgT1[:71, t, :],
                                w2[:71, 1, jj * 512:(jj + 1) * 512],
                                start=False, stop=True,
                            )
                        nc.scalar.copy(
                            ysb[:, h * (oo // 2):(h + 1) * (oo // 2), :], y[:])
                    nc.vector.tensor_tensor(
                        ysb[:], ysb[:],
                        enb[:, t, None, :].to_broadcast((P, oo, D)), OP.mult,
                    )
                    for w in (8, 4, 2):
                        nc.vector.tensor_tensor(
                            ysb[:, :, 0:w], ysb[:, :, 0:w], ysb[:, :, w:2 * w],
                            OP.add,
                        )
                    acc = wpool.tile([P, oo], f32, tag="acc")
                    nc.vector.tensor_tensor(acc[:], ysb[:, :, 0], ysb[:, :, 1],
                                            OP.add)

                    if phase == 0:
                        # z_r = sigmoid(acc); z = [:, :H], r = [:, H:]
                        zr = wpool.tile([P, 2 * H], f32, tag="zr")
                        nc.scalar.activation(zr[:], acc[:], AF.Sigmoid)
                        nc.vector.tensor_copy(z_f[:, t, :], zr[:, 0:H])
                        # X2 candidate cols 2:66 = r * state
                        nc.vector.tensor_mul(xg[:, t, CIN:C], zr[:, H:2 * H],
                                             st_f[:, t, :])
                    else:
                        hc = wpool.tile([P, H], f32, tag="hc")
                        nc.scalar.activation(hc[:], acc[:], AF.Tanh)
                        # h = state + z*(hc - state)
                        hd = wpool.tile([P, H], f32, tag="hd")
                        nc.vector.tensor_tensor(hd[:], hc[:], st_f[:, t, :],
                                                OP.subtract)
                        nc.vector.tensor_mul(hd[:], hd[:], z_f[:, t, :])
                        nc.vector.tensor_add(hd[:], hd[:], st_f[:, t, :])
                        nc.sync.dma_start(
                            out_d.ap().rearrange("(t p) h -> p t h", p=P)[:, t, :],
                            hd[:],
                        )
    nc.compile()
    return nc


def _prep_host(node_embeddings, gwp, gbp, uwp, ubp):
    # W2[(k,i),(d,o)] = pool[d,k,i,o]; append bias row; pad+chunk to (2,128,do)
    def pack(wp, bp, do):
        # columns ordered (o, d): w2[(k,i),(o,d)] = wp[d,k,i,o]
        w2 = np.transpose(wp, (1, 2, 3, 0)).reshape(3 * C, do)
        w2 = np.concatenate([w2, bp.T.reshape(1, do)], axis=0)  # (199, do)
        out = np.zeros((2, P, do), dtype=ml_dtypes.bfloat16)
        out[0, :, :] = w2[0:P].astype(ml_dtypes.bfloat16)
        out[1, 0:KI - P, :] = w2[P:KI].astype(ml_dtypes.bfloat16)
        return out
    w2g = pack(gwp, gbp, DOG)
    w2u = pack(uwp, ubp, DOU)
    return w2g, w2u


def kernel(x, state, node_embeddings, gate_weights_pool, gate_bias_pool,
           update_weights_pool, update_bias_pool):
    from concourse.bass_utils import run_bass_kernel_spmd

    if "nc" not in _CACHE:
        _CACHE["nc"] = _build()
    nc = _CACHE["nc"]

    w2g, w2u = _prep_host(node_embeddings, gate_weights_pool, gate_bias_pool,
                          update_weights_pool, update_bias_pool)
    emb = np.ascontiguousarray(node_embeddings, dtype=np.float32)
    in_maps = []
    for b in range(B):
        in_maps.append({
            "x": np.ascontiguousarray(x[b], dtype=np.float32),
            "state": np.ascontiguousarray(state[b], dtype=np.float32),
            "emb": emb,
            "idn": _EYE,
            "w2g": w2g,
            "w2u": w2u,
        })
    res = run_bass_kernel_spmd(nc, in_maps, core_ids=list(range(B)))
    out = np.stack([res.results[b]["out"] for b in range(B)], axis=0)
    return out.astype(np.float32)


# revision 21
# speedup vs baseline: 1.3538x; 1.3538x over previous
"""AGCRN cell Bass kernel for 8 TRN2 NeuronCores.

Sharding: data-parallel over batch (B=8 -> 1 batch per core). Zero collectives.

Math (per core, batch b):
  S_un = exp(relu(E @ E.T))           # symmetric! (elementwise funcs of symmetric EE^T)
  r    = 1 / rowsum(S_un)             # softmax normalization deferred to row scales
  S1 @ v = diag(r) @ (S_un @ v)       # S_un used as matmul weights directly (no transpose
                                      #   needed thanks to symmetry)
  Chebyshev by recurrence on the activation side:
    G1 = S1 @ X;  G2 = 2*S1@G1 - X    (never form S2 = 2 S1 S1 - I)
  Weight application without materializing per-node weights:
    y[n,(d,o)] = x_g[n,(k,i)] @ W2[(k,i),(d,o)],  W2[(k,i),(d,o)] = pool[d,k,i,o]
    out[n,o]   = sum_d E[n,d] * y[n,(d,o)] (+bias via augmented ones-row in x_g/W2)
  E-contraction on DVE via scalar_tensor_tensor with per-partition scalars E[n,d].
"""

import numpy as np
import ml_dtypes

B, N, D, CIN, H = 8, 2048, 16, 2, 64
C = CIN + H          # 66
KI = 3 * C + 1       # 199 = (k,i) rows + 1 bias ones-row
DOG = D * 2 * H      # 2048 gate (d,o) cols
DOU = D * H          # 1024 update
P = 128
NT = N // P          # 16

_CACHE = {}
_EYE = np.eye(P, dtype=ml_dtypes.bfloat16)


def _build():
    import concourse.bass as bass
    import concourse.tile as tile
    import concourse.mybir as mybir
    from concourse import bacc

    f32 = mybir.dt.float32
    bf16 = mybir.dt.bfloat16
    AF = mybir.ActivationFunctionType
    OP = mybir.AluOpType

    nc = bacc.Bacc(None, target_bir_lowering=False)

    x_d = nc.declare_dram_parameter("x", [N, CIN], f32, isOutput=False)
    st_d = nc.declare_dram_parameter("state", [N, H], f32, isOutput=False)
    e_d = nc.declare_dram_parameter("emb", [N, D], f32, isOutput=False)
    id_d = nc.declare_dram_parameter("idn", [P, P], bf16, isOutput=False)
    wg_d = nc.declare_dram_parameter("w2g", [2, P, DOG], bf16, isOutput=False)
    wu_d = nc.declare_dram_parameter("w2u", [2, P, DOU], bf16, isOutput=False)
    out_d = nc.declare_dram_parameter("out", [N, H], f32, isOutput=True)

    with tile.TileContext(nc) as tc:
        import contextlib
        ctx = contextlib.ExitStack()
        with ctx:
            cpool = ctx.enter_context(tc.tile_pool(name="const", bufs=1))
            wpool = ctx.enter_context(tc.tile_pool(name="work", bufs=4))
            pp = ctx.enter_context(tc.tile_pool(name="psA", bufs=2, space="PSUM"))
            ppy = ctx.enter_context(tc.tile_pool(name="psY", bufs=2, space="PSUM"))

            # ---- persistent SBUF ----
            S = cpool.tile([P, NT, N], bf16)          # S_un, row-tile t on axis1
            Et = cpool.tile([D, N], bf16)             # E^T (d on partitions)
            En = cpool.tile([P, NT, D], f32)          # E natural (per-partition scalars)
            rinv = cpool.tile([P, NT], f32)
            rinv2 = cpool.tile([P, NT], f32)
            rsum = cpool.tile([P, NT], f32)
            xg = cpool.tile([P, NT, 256], bf16)       # x_g node-major cols 0:199 (+pad)
            xgT0 = cpool.tile([P, NT, P], bf16)       # (x_g)^T rows 0:128
            xgT1 = cpool.tile([P, NT, P], bf16)       # (x_g)^T rows 128:199 (71 used)
            w2g = cpool.tile([P, 2, DOG], bf16)
            w2u = cpool.tile([P, 2, DOU], bf16)
            st_f = cpool.tile([P, NT, H], f32)
            x_f = cpool.tile([P, NT, CIN], f32)
            z_f = cpool.tile([P, NT, H], f32)
            idn = cpool.tile([P, P], bf16)
            idn_st = cpool.tile([P, P], bf16)
            enb = cpool.tile([P, NT, D], bf16)
            wg_st = cpool.tile([P, 2, DOG], bf16)
            wu_st = cpool.tile([P, 2, DOU], bf16)

            # ---- DMA inputs ----
            nc.sync.dma_start(x_f[:], x_d.ap().rearrange("(t p) c -> p t c", p=P))
            nc.sync.dma_start(st_f[:], st_d.ap().rearrange("(t p) h -> p t h", p=P))
            nc.sync.dma_start(En[:], e_d.ap().rearrange("(t p) d -> p t d", p=P))
            nc.sync.dma_start(wg_st[:], wg_d.ap().rearrange("c p f -> p c f"))
            nc.sync.dma_start(wu_st[:], wu_d.ap().rearrange("c p f -> p c f"))
            nc.sync.dma_start(idn_st[:], id_d.ap())
            # funnel all PE-visible tensors through DVE (matmul 1-wait limit)
            nc.vector.tensor_copy(w2g[:], wg_st[:])
            nc.vector.tensor_copy(w2u[:], wu_st[:])
            nc.vector.tensor_copy(idn[:], idn_st[:])
            nc.vector.tensor_copy(enb[:], En[:])

            # x_g cols 0:2 = x, ones col at 198
            nc.vector.tensor_copy(xg[:, :, 0:CIN], x_f[:])
            nc.vector.tensor_copy(xg[:, :, CIN:C], st_f[:])
            nc.vector.memset(xg[:, :, 198:199], 1.0)

            # ---- E^T via TensorE transpose (bf16) ----
            for t in range(NT):
                pt = pp.tile([D, P], bf16, tag="tp")
                nc.tensor.transpose(pt[:], enb[:, t, :], idn[:, :])
                nc.vector.tensor_copy(Et[:, t * P:(t + 1) * P], pt[:])

            # ---- S_un build: EE^T (bf16 matmul) -> exp (ACT) -> max(1,.)+rowsum (DVE) ----
            Etb = Et  # already bf16
            rsum2 = cpool.tile([P, NT, 2], f32)
            for t in range(NT):
                for h in range(2):
                    ee = ppy.tile([P, DOG // 2], f32, tag="y")
                    for j in range(2):
                        nc.tensor.matmul(
                            ee[:, j * 512:(j + 1) * 512],
                            Etb[:, t * P:(t + 1) * P],
                            Etb[:, (2 * h + j) * 512:(2 * h + j + 1) * 512],
                            start=True, stop=True,
                        )
                    etmp = wpool.tile([P, N // 2], bf16, tag="etmp")
                    nc.scalar.activation(etmp[:], ee[:], AF.Exp)
                    nc.vector.tensor_scalar(
                        S[:, t, h * (N // 2):(h + 1) * (N // 2)], etmp[:],
                        1.0, 0.0, OP.max, OP.add,
                        accum_out=rsum2[:, t, h:h + 1],
                    )
            nc.vector.tensor_tensor(rsum[:], rsum2[:, :, 0], rsum2[:, :, 1],
                                    OP.add)
            nc.vector.reciprocal(rinv[:], rsum[:])
            nc.vector.tensor_scalar_mul(rinv2[:], rinv[:], 2.0)

            # ---- two phases: gate (sigmoid -> z, r) and update (tanh -> hc -> h) ----
            for phase in range(2):
                dox = DOG if phase == 0 else DOU
                w2 = w2g if phase == 0 else w2u

                # app1: G1 = rinv * (S_un @ X)   -> xg cols 66:132
                for t in range(NT):
                    g1 = pp.tile([P, C], f32, tag="g")
                    for mc in range(NT):
                        nc.tensor.matmul(
                            g1[:], S[:, mc, t * P:(t + 1) * P], xg[:, mc, 0:C],
                            start=(mc == 0), stop=(mc == NT - 1),
                        )
                    nc.vector.tensor_scalar(
                        xg[:, t, C:2 * C], g1[:], rinv[:, t:t + 1], None, OP.mult,
                    )
                # app2: G2 = 2*rinv*(S_un @ G1) - X   -> xg cols 132:198
                for t in range(NT):
                    g2 = pp.tile([P, C], f32, tag="g")
                    for mc in range(NT):
                        nc.tensor.matmul(
                            g2[:], S[:, mc, t * P:(t + 1) * P], xg[:, mc, C:2 * C],
                            start=(mc == 0), stop=(mc == NT - 1),
                        )
                    nc.vector.scalar_tensor_tensor(
                        xg[:, t, 2 * C:3 * C], g2[:], rinv2[:, t:t + 1],
                        xg[:, t, 0:C], OP.mult, OP.subtract,
                    )

                for t in range(NT):
                    # transpose x_g tile -> (ki, n) chunks
                    pt0 = pp.tile([P, P], bf16, tag="tp")
                    nc.tensor.transpose(pt0[:], xg[:, t, 0:P], idn[:, :])
                    nc.vector.tensor_copy(xgT0[:, t, :], pt0[:])
                    pt1 = pp.tile([71, P], bf16, tag="tp")
                    nc.tensor.transpose(pt1[:], xg[:, t, P:KI], idn[:, :])
                    nc.vector.tensor_copy(xgT1[:71, t, :], pt1[:])

                    # weight matmul in halves: ACT copy of half h overlaps
                    # PE matmul of half h+1 (separate psum tiles, bufs=2)
                    oo = dox // D  # 128 gate, 64 update
                    ysb = wpool.tile([P, oo, D], bf16, tag="ysb")
                    hw_ = dox // 2
                    for h in range(2):
                        y = ppy.tile([P, hw_], f32, tag="y")
                        for j in range(hw_ // 512):
                            jj = h * (hw_ // 512) + j
                            nc.tensor.matmul(
                                y[:, j * 512:(j + 1) * 512], xgT0[:, t, :],
                                w2[:, 0, jj * 512:(jj + 1) * 512],
                                start=True, stop=False,
                            )
                            nc.tensor.matmul(
                                y[:, j * 512:(j + 1) * 512], xgT1[:71, t, :],
                                w2[:71, 1, jj * 512:(jj + 1) * 512],
                                start=False, stop=True,
                            )
                        nc.scalar.copy(
                            ysb[:, h * (oo // 2):(h + 1) * (oo // 2), :], y[:])
                    nc.vector.tensor_tensor(
                        ysb[:], ysb[:],
                        enb[:, t, None, :].to_broadcast((P, oo, D)), OP.mult,
                    )
                    for w in (8, 4, 2):
                        nc.vector.tensor_tensor(
                            ysb[:, :, 0:w], ysb[:, :, 0:w], ysb[:, :, w:2 * w],
                            OP.add,
                        )
                    acc = wpool.tile([P, oo], f32, tag="acc")
                    nc.vector.tensor_tensor(acc[:], ysb[:, :, 0], ysb[:, :, 1],
                                            OP.add)

                    if phase == 0:
                        # z_r = sigmoid(acc); z = [:, :H], r = [:, H:]
                        zr = wpool.tile([P, 2 * H], f32, tag="zr")
                        nc.scalar.activation(zr[:], acc[:], AF.Sigmoid)
                        nc.vector.tensor_copy(z_f[:, t, :], zr[:, 0:H])
                        # X2 candidate cols 2:66 = r * state
                        nc.vector.tensor_mul(xg[:, t, CIN:C], zr[:, H:2 * H],
                                             st_f[:, t, :])
                    else:
                        hc = wpool.tile([P, H], f32, tag="hc")
                        nc.scalar.activation(hc[:], acc[:], AF.Tanh)
                        # h = state + z*(hc - state)
                        hd = wpool.tile([P, H], f32, tag="hd")
                        nc.vector.tensor_tensor(hd[:], hc[:], st_f[:, t, :],
                                                OP.subtract)
                        nc.vector.tensor_mul(hd[:], hd[:], z_f[:, t, :])
                        nc.vector.tensor_add(hd[:], hd[:], st_f[:, t, :])
                        nc.sync.dma_start(
                            out_d.ap().rearrange("(t p) h -> p t h", p=P)[:, t, :],
                            hd[:],
                        )
    nc.compile()
    return nc


def _prep_host(node_embeddings, gwp, gbp, uwp, ubp):
    # W2[(k,i),(d,o)] = pool[d,k,i,o]; append bias row; pad+chunk to (2,128,do)
    def pack(wp, bp, do):
        # columns ordered (o, d): w2[(k,i),(o,d)] = wp[d,k,i,o]
        w2 = np.transpose(wp, (1, 2, 3, 0)).reshape(3 * C, do)
        w2 = np.concatenate([w2, bp.T.reshape(1, do)], axis=0)  # (199, do)
        out = np.zeros((2, P, do), dtype=ml_dtypes.bfloat16)
        out[0, :, :] = w2[0:P].astype(ml_dtypes.bfloat16)
        out[1, 0:KI - P, :] = w2[P:KI].astype(ml_dtypes.bfloat16)
        return out
    w2g = pack(gwp, gbp, DOG)
    w2u = pack(uwp, ubp, DOU)
    return w2g, w2u


def kernel(x, state, node_embeddings, gate_weights_pool, gate_bias_pool,
           update_weights_pool, update_bias_pool):
    from concourse.bass_utils import run_bass_kernel_spmd

    if "nc" not in _CACHE:
        _CACHE["nc"] = _build()
    nc = _CACHE["nc"]

    w2g, w2u = _prep_host(node_embeddings, gate_weights_pool, gate_bias_pool,
                          update_weights_pool, update_bias_pool)
    emb = np.ascontiguousarray(node_embeddings, dtype=np.float32)
    in_maps = []
    for b in range(B):
        in_maps.append({
            "x": np.ascontiguousarray(x[b], dtype=np.float32),
            "state": np.ascontiguousarray(state[b], dtype=np.float32),
            "emb": emb,
            "idn": _EYE,
            "w2g": w2g,
            "w2u": w2u,
        })
    res = run_bass_kernel_spmd(nc, in_maps, core_ids=list(range(B)))
    out = np.stack([res.results[b]["out"] for b in range(B)], axis=0)
    return out.astype(np.float32)
